# revision 7
# baseline (speedup 1.0000x reference)
"""Soft Hausdorff loss kernel for Trainium2 (8 NeuronCores).

Reference computation (per batch b, N=M=4096, d=3):
    dist[n,m] = ||p1[n] - p2[m]||^2
    loss = (mean_b lse(A*min_m dist)/A + mean_b lse(A*min_n dist)/A) / 2,  A=10

Device strategy (per core: one (batch, n-half) pair -> 2048 rows x 4096 cols):
  * dist tiles [128,512] are produced straight into PSUM by one K=24 matmul:
    every f32 operand is split into 3 bf16 terms (exact to ~2^-27), and the
    contraction rows hold the coordinate cross products (6 of 9 split combos),
    sq1 (triple-split) x 1, and 1 x sq2 (triple-split). bf16 matmul streams
    1 column/cycle (fp32 would cost 4x).
  * row/col minima come from a softmin: ScalarE computes exp(-BETA*dist) from
    PSUM with a fused per-row accumulation (row sums), writing the exp tile to
    SBUF in bf16; TensorE then multiplies by a ones vector to accumulate
    column sums in PSUM. -log(S)/BETA recovers minima to ~ln(k)/BETA.
  * the host finishes in f64: combines partial sums, takes logs, and exactly
    recomputes the few rows/cols whose softmin is unreliable (large minima
    underflow exp; they are rare and cheap to redo in numpy).
"""

import os
from contextlib import ExitStack

import ml_dtypes
import numpy as np

import concourse.bass as bass
import concourse.bacc as bacc
import concourse.tile as tile
from concourse import mybir
from concourse.bass_utils import run_bass_kernel_spmd

BF16 = ml_dtypes.bfloat16

ALPHA = 10.0
BETA = 300.0
FALLBACK_T = 0.25  # recompute rows/cols whose estimated min exceeds this

B, N, D = 4, 4096, 3
HALF = N // 2  # rows handled per core
NCH = HALF // 128  # 16 chunks of 128 rows
NW = N // 512  # 8 column windows
NG = NW // 2  # 4 activation groups (free dim 1024)
K = 24  # contraction rows of the augmented matmul

# which (p1-split, p2-split) products to keep; dropping (1,2),(2,1),(2,2)
# leaves ~2^-27 relative error on the inner product
COMBOS = [(0, 0), (0, 1), (1, 0), (0, 2), (2, 0), (1, 1)]

_NC_CACHE = {}


def _build_nc(do_compile=True):
    nc = bacc.Bacc(trn_type="TRN2")
    a1 = nc.dram_tensor("a1", [K, HALF], mybir.dt.bfloat16, kind="ExternalInput")
    a2 = nc.dram_tensor("a2", [K, N], mybir.dt.bfloat16, kind="ExternalInput")
    rs = nc.dram_tensor("rs", [128, NCH * NG], mybir.dt.float32, kind="ExternalOutput")
    cm = nc.dram_tensor("cm", [NW, 512], mybir.dt.float32, kind="ExternalOutput")

    with ExitStack() as ctx:
        tc = ctx.enter_context(tile.TileContext(nc))
        const = ctx.enter_context(tc.tile_pool(name="const", bufs=1))
        epool = ctx.enter_context(tc.tile_pool(name="epool", bufs=3))
        pdist = ctx.enter_context(tc.tile_pool(name="pdist", bufs=3, space="PSUM"))
        pcm = ctx.enter_context(tc.tile_pool(name="pcm", bufs=1, space="PSUM"))
        outp = ctx.enter_context(tc.tile_pool(name="outp", bufs=1))

        a1_sb = const.tile([K, HALF], mybir.dt.bfloat16)
        nc.sync.dma_start(out=a1_sb, in_=a1[:, :])
        a2_sb = const.tile([K, N], mybir.dt.bfloat16)
        nc.sync.dma_start(out=a2_sb, in_=a2[:, :])
        ones_sb = const.tile([128, 1], mybir.dt.bfloat16)
        nc.vector.memset(ones_sb, 1.0)

        rs_sb = outp.tile([128, NCH * NG], mybir.dt.float32)
        cm_acc = [
            pcm.tile([128, 512], mybir.dt.float32, name=f"cmacc{i}", tag=f"cmacc{i}")
            for i in range(2)
        ]
        # the ones-matmuls only write partitions {0,32,64,96}; initialize the
        # rest so the final full-bank copies read defined memory. Done on ACT
        # so the first ones-matmul's waits collapse onto one semaphore
        # (walrus allows a single sync wait per matmul).
        nc.scalar.memzero(cm_acc[0])
        nc.scalar.memzero(cm_acc[1])

        for c in range(NCH):
            lhsT = a1_sb[:, c * 128 : (c + 1) * 128]
            for g in range(NG):
                pd = pdist.tile([128, 2, 512], mybir.dt.float32)
                for s in range(2):
                    w = g * 2 + s
                    nc.tensor.matmul(
                        pd[:, s, :],
                        lhsT,
                        a2_sb[:, w * 512 : (w + 1) * 512],
                        start=True,
                        stop=True,
                    )
                eg = epool.tile([128, 2, 512], mybir.dt.bfloat16)
                k = c * NG + g
                nc.scalar.activation(
                    out=eg,
                    in_=pd,
                    func=mybir.ActivationFunctionType.Exp,
                    scale=-BETA,
                    accum_out=rs_sb[:, k : k + 1],
                )
                for s in range(2):
                    w = g * 2 + s
                    bank = cm_acc[w // 4]
                    j = w % 4
                    nc.tensor.matmul(
                        bank[32 * j : 32 * j + 1, :],
                        ones_sb,
                        eg[:, s, :],
                        start=(c == 0),
                        stop=(c == NCH - 1),
                        tile_position=(0, 32 * j),
                    )

        cmA_sb = outp.tile([128, 512], mybir.dt.float32, tag="cmA_sb")
        cmB_sb = outp.tile([128, 512], mybir.dt.float32, tag="cmB_sb")
        nc.vector.tensor_copy(out=cmA_sb, in_=cm_acc[0])
        nc.vector.tensor_copy(out=cmB_sb, in_=cm_acc[1])
        nc.sync.dma_start(out=rs[:, :], in_=rs_sb)
        for j in range(4):
            nc.sync.dma_start(out=cm[j : j + 1, :], in_=cmA_sb[32 * j : 32 * j + 1, :])
            nc.sync.dma_start(
                out=cm[4 + j : 5 + j, :], in_=cmB_sb[32 * j : 32 * j + 1, :]
            )
    if do_compile:
        nc.compile()
    return nc


def _get_nc():
    if "nc" not in _NC_CACHE:
        _NC_CACHE["nc"] = _build_nc()
    return _NC_CACHE["nc"]


def _split3(x):
    """f32 array -> three bf16 arrays summing to x (exact to ~2^-27 rel)."""
    x0 = np.asarray(x, dtype=np.float32)
    h = x0.astype(BF16)
    r = x0 - h.astype(np.float32)
    m = r.astype(BF16)
    l = (r - m.astype(np.float32)).astype(BF16)
    return h, m, l


def _prep_core_inputs(p1b, p2b, half):
    """Build the [K, HALF] stationary-side and [K, N] moving-side bf16 arrays."""
    a = p1b[half * HALF : (half + 1) * HALF]
    b = p2b
    asp = _split3(a)  # each [HALF, 3]
    bsp = _split3(b)  # each [N, 3]
    s1 = _split3((a.astype(np.float64) ** 2).sum(-1))
    s2 = _split3((b.astype(np.float64) ** 2).sum(-1))

    a1 = np.zeros([K, HALF], dtype=BF16)
    a2 = np.zeros([K, N], dtype=BF16)
    r = 0
    for d in range(D):
        for (i, j) in COMBOS:
            a1[r] = asp[i][:, d]
            # exact: bf16 scaled by a power of two
            a2[r] = (bsp[j][:, d].astype(np.float32) * np.float32(-2.0)).astype(BF16)
            r += 1
    for t in range(3):
        a1[r] = s1[t]
        a2[r] = np.float32(1.0)
        r += 1
    for t in range(3):
        a1[r] = np.float32(1.0)
        a2[r] = s2[t]
        r += 1
    assert r == K
    return a1, a2


def _install_ntff_hook_shim():
    """Provide antenv.axon_hooks (absent in this image) so that
    run_bass_kernel_spmd(trace=True) can capture NTFF profiles through the
    axon tunnel; also neuter the S3 artifact upload (no egress here)."""
    import sys
    import types

    if "antenv.axon_hooks" not in sys.modules:
        mod = types.ModuleType("antenv.axon_hooks")
        mod._hook = None
        mod.set_axon_ntff_profile_hook = lambda h: setattr(mod, "_hook", h)
        mod.get_axon_ntff_profile_hook = lambda: mod._hook
        sys.modules["antenv.axon_hooks"] = mod
        import antenv

        antenv.axon_hooks = mod

    import antenv.axon_hooks as ah

    if ah.get_axon_ntff_profile_hook() is None:
        from trn_agent_boot.trn_boot import _ntff_profile_via_ctypes

        so_path = "/opt/axon/libaxon_pjrt.so"
        hook = _ntff_profile_via_ctypes(so_path)
        if hook is not None:
            ah.set_axon_ntff_profile_hook(hook)

    import concourse.bass_utils as bu

    bu.upload_artifacts = lambda tmpdir: str(tmpdir)


def _logsumexp(x, axis):
    mx = np.max(x, axis=axis, keepdims=True)
    return (mx + np.log(np.sum(np.exp(x - mx), axis=axis, keepdims=True))).squeeze(axis)


def kernel(p1, p2):
    p1 = np.ascontiguousarray(np.asarray(p1), dtype=np.float32)
    p2 = np.ascontiguousarray(np.asarray(p2), dtype=np.float32)
    nc = _get_nc()

    in_maps = []
    for core in range(8):
        b, half = core // 2, core % 2
        a1, a2 = _prep_core_inputs(p1[b], p2[b], half)
        in_maps.append({"a1": a1, "a2": a2})

    trace = bool(int(os.environ.get("HAUS_TRACE", "0")))
    if trace:
        _install_ntff_hook_shim()
    res = run_bass_kernel_spmd(nc, in_maps, core_ids=list(range(8)), trace=trace)
    if trace and res.exec_time_ns is not None:
        print(f"HW exec time: {res.exec_time_ns} ns")
        if res.instructions_and_trace is not None:
            print("trace:", res.instructions_and_trace[1])
    results = res.results

    rmins = np.zeros((B, N))
    cmins = np.zeros((B, N))
    for b in range(B):
        for half in range(2):
            r = results[2 * b + half]
            S = r["rs"].astype(np.float64).reshape(128, NCH, NG).sum(-1)  # [128, NCH]
            m = -np.log(np.maximum(S, 1e-300)) / BETA
            rmins[b, half * HALF : (half + 1) * HALF] = m.T.reshape(-1)
        Sc = results[2 * b]["cm"].astype(np.float64) + results[2 * b + 1]["cm"].astype(
            np.float64
        )
        cmins[b] = (-np.log(np.maximum(Sc, 1e-300)) / BETA).reshape(-1)

    # exact host fallback for unreliable (large-min) rows/cols
    for b in range(B):
        p1b = p1[b].astype(np.float64)
        p2b = p2[b].astype(np.float64)
        bad_r = np.nonzero(~np.isfinite(rmins[b]) | (rmins[b] > FALLBACK_T))[0]
        if bad_r.size:
            d = ((p1b[bad_r, None, :] - p2b[None, :, :]) ** 2).sum(-1)
            rmins[b, bad_r] = d.min(1)
        bad_c = np.nonzero(~np.isfinite(cmins[b]) | (cmins[b] > FALLBACK_T))[0]
        if bad_c.size:
            d = ((p1b[:, None, :] - p2b[None, bad_c, :]) ** 2).sum(-1)
            cmins[b, bad_c] = d.min(0)

    r_ls = _logsumexp(ALPHA * rmins, axis=1) / ALPHA
    c_ls = _logsumexp(ALPHA * cmins, axis=1) / ALPHA
    val = (r_ls.mean() + c_ls.mean()) / 2.0
    return np.float32(val)
